# revision 13
# baseline (speedup 1.0000x reference)
"""Causal multi-head attention on 8 Trainium2 NeuronCores.

Problem: x[4,2048,1024] @ w_qkv[1024,3072] -> causal MHA (16 heads, hd=64) -> @ w_out.

Sharding: batch (4) x head-group (2 x 8 heads) = 8 cores. Each core:
  phase 1: QKV projection for its batch + its 8 heads.
           Q^T,K^T produced transposed [ch, t] (lhsT = w chunk, rhs = x^T chunk),
           V produced natural [t, ch] (lhsT = x^T chunk, rhs = w_v chunk),
           with a ones column appended per head ([V|1]) for the softmax denominator.
  phase 2: causal attention per head. S^T[k,q] = K_h^T(.T) @ Q_h^T per 128-key block
           (only non-fully-masked blocks computed), additive -1e9 triangular mask on
           diagonal blocks, exp on ACT (softmax max-subtraction skipped: |s/8| <~ 10),
           attn^T[d,q] + denom accumulated in PSUM via lhsT=[V|1], normalized by
           reciprocal + gpsimd partition_broadcast + DVE multiply.
  phase 3: partial output projection out = attn^T.T @ w_out (rows of w_out owned by
           this head group). Host sums the two partials per batch (2-way reduction).

All matmuls run in bf16 (full PE rate, FWL fast weight load; fp32r serialized
per-MM LDWEIGHTS at ~230ns and kept HAM cold: 450us PE active vs ~265us theoretical).
"""
import sys

if "/opt/trn_rl_repo" not in sys.path:
    sys.path.insert(0, "/opt/trn_rl_repo")

import ml_dtypes
import numpy as np

import concourse.tile as tile
from concourse import bacc, mybir
from concourse.bass_utils import run_bass_kernel_spmd

F32 = mybir.dt.float32
F32R = mybir.dt.float32r
BF16 = mybir.dt.bfloat16
EXP = mybir.ActivationFunctionType.Exp

B, T, C, H = 4, 2048, 1024, 16
HD = C // H              # 64
HPC = 8                  # heads per core
CPC = HPC * HD           # 512 channels per core
NCHUNK = C // 128        # 8 contraction chunks of 128
NQ = 4                   # t-quarters (512 each) for phase-1 x streaming
TQ = T // NQ             # 512
NKB = T // 128           # 16 key blocks
NCT = CPC // 128         # 4 c'-tiles per projection (q and k each)

_NC_CACHE = None


def _build_nc():
    """Build the SPMD program (identical on all 8 cores)."""
    nc = bacc.Bacc()

    wqkv = nc.dram_tensor("wqkv", [NCHUNK, 128, 3 * CPC], BF16, kind="ExternalInput")
    xq = nc.dram_tensor("xq", [NQ, NCHUNK, 128, TQ], BF16, kind="ExternalInput")
    wo = nc.dram_tensor("wo", [NCT, 128, C], BF16, kind="ExternalInput")
    maskneg = nc.dram_tensor("maskneg", [128, 128], BF16, kind="ExternalInput")
    out = nc.dram_tensor("out", [T, C], F32, kind="ExternalOutput")

    with tile.TileContext(nc) as tc:
        with tc.tile_pool(name="pers", bufs=1) as pers:
            # persistent SBUF
            qkt = [pers.tile([128, T], BF16, name=f"qkt{i}") for i in range(2 * NCT)]
            vsb = pers.tile([128, NKB * (CPC + HPC)], BF16, name="vsb")  # 16 x (8 x 65)
            mask_sb = pers.tile([128, 128], F32, name="mask_sb")
            nc.sync.dma_start(mask_sb[:], maskneg[:, :])
            # ones columns of [V|1]: memset f32 staging, strided DVE copy (cast to f32r)
            ones_sb = pers.tile([128, NKB * HPC], F32, name="ones_sb")
            nc.vector.memset(ones_sb[:], 1.0)
            nc.vector.tensor_copy(
                vsb.rearrange("p (t h e) -> p (t h) e", h=HPC, e=HD + 1)[:, :, HD:HD + 1],
                ones_sb[:, :, None],
            )

            # ---------------- phase 1: QKV projection ----------------
            with tc.tile_pool(name="wpool", bufs=1) as wpool, \
                 tc.tile_pool(name="xpool", bufs=3) as xpool, \
                 tc.tile_pool(name="ps1", bufs=3, space="PSUM") as ps1:
                w_sb = []
                for c in range(NCHUNK):
                    wt = wpool.tile([128, 3 * CPC], BF16, name=f"w{c}")
                    nc.sync.dma_start(wt[:], wqkv[c])
                    w_sb.append(wt)
                for tq in range(NQ):
                    xt = []
                    for c in range(NCHUNK):
                        x_t = xpool.tile([128, TQ], BF16, name=f"x{c}", tag=f"x{c}")
                        nc.sync.dma_start(x_t[:], xq[tq, c])
                        xt.append(x_t)
                    # Q^T (ct 0-3) and K^T (ct 4-7): out [c'128, t512]
                    for ct in range(2 * NCT):
                        wcol = 128 * ct  # q cols then k cols, contiguous in wqkv packing
                        ps = ps1.tile([128, TQ], F32, name="psqk", tag="ps1")
                        for c in range(NCHUNK):
                            nc.tensor.matmul(
                                ps[:], w_sb[c][:, wcol:wcol + 128], xt[c][:],
                                start=(c == 0), stop=(c == NCHUNK - 1),
                            )
                        nc.vector.tensor_copy(qkt[ct][:, TQ * tq:TQ * (tq + 1)], ps[:])
                    # V natural: out [t128, 512ch]
                    for vt in range(TQ // 128):
                        ps = ps1.tile([128, CPC], F32, name="psv", tag="ps1")
                        for c in range(NCHUNK):
                            nc.tensor.matmul(
                                ps[:], xt[c][:, 128 * vt:128 * (vt + 1)],
                                w_sb[c][:, 2 * CPC:3 * CPC],
                                start=(c == 0), stop=(c == NCHUNK - 1),
                            )
                        ti = tq * (TQ // 128) + vt
                        dst = vsb[:, (CPC + HPC) * ti:(CPC + HPC) * (ti + 1)]
                        nc.vector.tensor_copy(
                            dst.rearrange("p (h e) -> p h e", e=HD + 1)[:, :, 0:HD],
                            ps.rearrange("p (h e) -> p h e", e=HD),
                        )

            # ---------------- phase 2: causal attention ----------------
            atn = [pers.tile([128, T], BF16, name=f"atn{i}") for i in range(NCT)]
            with tc.tile_pool(name="epool", bufs=3) as epool, \
                 tc.tile_pool(name="npool", bufs=4) as npool, \
                 tc.tile_pool(name="psS", bufs=3, space="PSUM") as psS, \
                 tc.tile_pool(name="psA", bufs=2, space="PSUM") as psA:
                for h in range(HPC):
                    ct, r0 = h // 2, HD * (h % 2)
                    Qh = qkt[ct][r0:r0 + HD, :]
                    Kh = qkt[NCT + ct][r0:r0 + HD, :]
                    for j in range(4):  # 512-query tiles
                        q0 = 512 * j
                        nkb = 4 * j + 4  # key blocks: 4j full + 4 diagonal partials
                        pa = psA.tile([HD + 1, 512], F32, name="pa", tag="A")
                        for kbp in range((nkb + 1) // 2):
                            kbs = [k for k in (2 * kbp, 2 * kbp + 1) if k < nkb]
                            segs = []  # (kb, col0, N, off)
                            off = 0
                            for kb in kbs:
                                col0 = 0 if kb < 4 * j else 128 * (kb - 4 * j)
                                n = 512 - col0
                                segs.append((kb, col0, n, off))
                                off += n
                            wsum = off
                            ss = psS.tile([128, 1024], F32, name="ss", tag="S")
                            ee = epool.tile([128, 1024], BF16, name="ee", tag="E")
                            for kb, col0, n, off in segs:
                                nc.tensor.matmul(
                                    ss[:, off:off + n],
                                    Kh[:, 128 * kb:128 * (kb + 1)],
                                    Qh[:, q0 + col0:q0 + 512],
                                    start=True, stop=True, skip_group_check=True,
                                )
                                if kb >= 4 * j:  # diagonal: additive causal mask
                                    nc.vector.tensor_add(
                                        ss[:, off:off + 128], ss[:, off:off + 128], mask_sb[:]
                                    )
                            nc.scalar.activation(ee[:, 0:wsum], ss[:, 0:wsum], EXP, scale=0.125)
                            for kb, col0, n, off in segs:
                                nc.tensor.matmul(
                                    pa[:, col0:512],
                                    vsb[:, (CPC + HPC) * kb + (HD + 1) * h:
                                         (CPC + HPC) * kb + (HD + 1) * (h + 1)],
                                    ee[:, off:off + n],
                                    start=(kb == 0), stop=(kb == nkb - 1),
                                    skip_group_check=True,
                                )
                        # normalize by the denominator row
                        rec = npool.tile([1, 512], F32, name="rec", tag="rec")
                        nc.vector.reciprocal_approx_fast(rec[:], pa[HD:HD + 1, :])
                        bc = npool.tile([HD, 512], F32, name="bc", tag="bc")
                        nc.gpsimd.partition_broadcast(bc[:], rec[:])
                        nc.vector.tensor_mul(
                            atn[ct][r0:r0 + HD, q0:q0 + 512], pa[0:HD, :], bc[:]
                        )

            # ---------------- phase 3: output projection (partial) ----------------
            with tc.tile_pool(name="wopool", bufs=1) as wopool, \
                 tc.tile_pool(name="opool", bufs=4) as opool, \
                 tc.tile_pool(name="psO", bufs=3, space="PSUM") as psO:
                wo_sb = []
                for cc in range(NCT):
                    wt = wopool.tile([128, C], BF16, name=f"wo{cc}")
                    nc.sync.dma_start(wt[:], wo[cc])
                    wo_sb.append(wt)
                for tt in range(T // 128):
                    for jj in range(C // 512):
                        ps = psO.tile([128, 512], F32, name="po", tag="O")
                        for cc in range(NCT):
                            nc.tensor.matmul(
                                ps[:], atn[cc][:, 128 * tt:128 * (tt + 1)],
                                wo_sb[cc][:, 512 * jj:512 * (jj + 1)],
                                start=(cc == 0), stop=(cc == NCT - 1),
                            )
                        oc = opool.tile([128, 512], F32, name="oc", tag="oc")
                        nc.vector.tensor_copy(oc[:], ps[:])
                        nc.gpsimd.dma_start(
                            out[128 * tt:128 * (tt + 1), 512 * jj:512 * (jj + 1)], oc[:]
                        )
    nc.finalize()
    return nc


def _prep_inputs(x, w_qkv, w_out):
    """Shard + pack host-side: returns in_maps for cores 0..7 (core = 2*b + g)."""
    in_maps = []
    maskneg = np.where(
        np.arange(128)[None, :] >= np.arange(128)[:, None], 1.0, 0.0
    ).astype(ml_dtypes.bfloat16)
    for b in range(B):
        xT = np.ascontiguousarray(x[b].T)  # [C, T]
        xq_bf = np.ascontiguousarray(
            xT.reshape(NCHUNK, 128, NQ, TQ).transpose(2, 0, 1, 3)
        ).astype(ml_dtypes.bfloat16)  # [NQ, NCHUNK, 128, TQ]
        for g in range(2):
            wq = w_qkv[:, CPC * g:CPC * (g + 1)]
            wk = w_qkv[:, C + CPC * g:C + CPC * (g + 1)]
            wv = w_qkv[:, 2 * C + CPC * g:2 * C + CPC * (g + 1)]
            wqkv_pack = np.concatenate([wq, wk, wv], axis=1).reshape(
                NCHUNK, 128, 3 * CPC
            )
            wo_pack = np.ascontiguousarray(
                w_out[CPC * g:CPC * (g + 1), :].reshape(NCT, 128, C)
            )
            in_maps.append({
                "wqkv": np.ascontiguousarray(wqkv_pack).astype(ml_dtypes.bfloat16),
                "xq": xq_bf,
                "wo": wo_pack.astype(ml_dtypes.bfloat16),
                "maskneg": maskneg,
            })
    return in_maps


def run(x, w_qkv, w_out, trace=False, trace_cores=None):
    global _NC_CACHE
    if _NC_CACHE is None:
        _NC_CACHE = _build_nc()
    in_maps = _prep_inputs(x, w_qkv, w_out)
    res = run_bass_kernel_spmd(
        _NC_CACHE, in_maps, list(range(8)),
        trace=trace, trace_cores=trace_cores,
    )
    outs = [res.results[i]["out"] for i in range(8)]
    full = np.empty((B, T, C), np.float32)
    for b in range(B):
        full[b] = outs[2 * b] + outs[2 * b + 1]
    return full, res


def kernel(x, w_qkv, w_out):
    x = np.asarray(x, np.float32)
    w_qkv = np.asarray(w_qkv, np.float32)
    w_out = np.asarray(w_out, np.float32)
    full, _ = run(x, w_qkv, w_out)
    return full
